# revision 27
# baseline (speedup 1.0000x reference)
"""Trainium2 Bass kernel for nn_DMLoss (contour matching loss), 8-core data parallel.

v6: per-instance K=10 matmuls (no block-diag zeros, no backfill), 6 bulk
operand reloads via DRAM bounce, critical-chain-first issue order, fused
clamp-in-relu rounding, engine-balanced main loop, XBARs on sync queue.

Per instance (P=128 points, TIME=10):
  item1: nearest of 1280 interpolated gt points per pred point.  Segment n
    spans A_n = gt[n-1]..gt[n]; g' = 10*u - 0.5 with u = <p-A,D>/|D|^2; best
    discrete t = clamp(round(g'), 0, 9); dist^2 = |p-A|^2 + (e/100)*t*(t-2g').
    TensorE produces grids g', opsC1 = SC*(2<p,A> - |A|^2) + C1 and
    erep = -SC*e/100 (bf16 hi/lo split rows, C1 = C1H + C1L exactly).
    dq = (q*erep + opsC1) - CQ lands exactly on the 128-grid; pk = dq - n
    packs quantized distance + segment index; reduce-max = argmin;
    exact 0/1 one-hots gather segment data via XBAR transpose + bf16 matmul.
  item2: same machinery without interpolation (nearest pred per key point).

Output per core: [sum_loss1, sum_loss2]; host divides by counts and combines.
"""
import sys

for _p in ("/opt/trn_rl_repo",):
    if _p not in sys.path:
        sys.path.insert(0, _p)

import numpy as np

import concourse.bass as bass
import concourse.tile as tile
from concourse import bacc, mybir
from concourse.bass_utils import run_bass_kernel_spmd

dt = mybir.dt
Alu = mybir.AluOpType
Ax = mybir.AxisListType
Act = mybir.ActivationFunctionType
f32 = np.float32

N_CORES = 8
N, P = 256, 128
G = N // N_CORES          # instances per core = 32
BG = 4                    # instances per block
NB = G // BG              # 8 blocks
SC = 131072.0             # distance scale (quantum = 128/SC = 2^-10)
SHIFT = 48.0
BETA = 0.25               # smooth-l1 beta = 1/STRIDE
CQ = float(2 ** 30 + 2 ** 25)       # 1107296256
C1 = CQ - SC * SHIFT                # 1101004800
C1H = 1098907648.0                  # bf16-exact hi part of C1
C1L = 2097152.0                     # bf16-exact lo part (C1H + C1L == C1)
M23 = 8388608.0

# Slab row types (k = row index 0..9 within an instance):
#   lhsT: [x_hi, y_hi, x_lo, y_lo, x_hi2, y_hi2, 1, 1, 1, 1]
#   rhs:  [u_hi, v_hi, u_hi2, v_hi2, u_lo, v_lo, c_hi, c_lo, w8, w9]
# pairing k: xh*uh + yh*vh + xl*uh + yl*vh + xh*ul + yh*vl + ch + cl + w8 + w9
P0 = 0
K0 = 10


def _build(nc, pc_d, po_d, gc_d, gk_d, mk_d, out_d):
    FP = dt.float32
    BF = dt.bfloat16

    with tile.TileContext(nc) as tc:
        with (
            tc.tile_pool(name="const", bufs=1) as cpool,
            tc.tile_pool(name="prep", bufs=1) as prep,
            tc.tile_pool(name="oper", bufs=1) as oper,
            tc.tile_pool(name="main", bufs=3) as main,
            tc.tile_pool(name="keep", bufs=1) as keep,
        ):
            V, Gp, S = nc.vector, nc.gpsimd, nc.scalar

            # ---------------- input loads (sync queue, gc first) -------------
            pc_i = prep.tile([32, 128, 2], FP, tag="pc_i")
            po_i = prep.tile([32, 128, 2], FP, tag="po_i")
            gc_i = prep.tile([32, 128, 2], FP, tag="gc_i")
            gk_i = prep.tile([32, 128, 2], FP, tag="gk_i")
            mk_i = prep.tile([32, 128], FP, tag="mk_i")
            a_i = prep.tile([32, 128, 2], FP, tag="a_i")
            nc.sync.dma_start(gc_i[:], gc_d[:, :, :])
            nc.sync.dma_start(pc_i[:], pc_d[:, :, :])
            nc.sync.dma_start(gk_i[:], gk_d[:, :, :])
            nc.sync.dma_start(po_i[:], po_d[:, :, :])
            nc.sync.dma_start(mk_i[:], mk_d[:, :])

            m23n = cpool.tile([128, 1], FP, tag="m23n")
            Gp.memset(m23n[:], -M23)

            # zero source for rhs off-diagonal backfill; memset before any
            # dependent compute so the backfill DMAs fire immediately.
            lhsA = oper.tile([40, 2, 8, 128], BF, tag="lhsA")
            rhsA = oper.tile([40, 8, 4, 512], BF, tag="rhsA")
            ztile = prep.tile([40, 4, 512], BF, tag="ztile")
            Gp.memset(ztile[:], 0.0)
            for j in range(8):
                eng = nc.scalar if (j % 2 == 0) else nc.sync
                eng.dma_start(
                    rhsA[:].rearrange("t b r n -> t (b r) n")[:, 4 * j:4 * j + 4, :],
                    ztile[:],
                )

            # ---------------- slab staging tiles -----------------------------
            sPK = prep.tile([32, 20, 128], BF, tag="sPK")
            sG = prep.tile([32, 10, 128], BF, tag="sG")
            sO = prep.tile([32, 10, 128], BF, tag="sO")
            sQ = prep.tile([32, 10, 128], BF, tag="sQ")
            sE = prep.tile([32, 10, 128], BF, tag="sE")
            sTB = prep.tile([32, 17, 128], BF, tag="sTB")
            # const rows filled up-front on the idle Vector queue
            V.memset(sPK[:, P0 + 6:P0 + 10, :], 1.0)
            V.memset(sPK[:, K0 + 6:K0 + 10, :], 1.0)
            V.memset(sQ[:, 8, :], C1H)
            V.memset(sQ[:, 9, :], C1L)
            V.memset(sE[:, 0:6, :], 0.0)
            V.memset(sE[:, 8:10, :], 0.0)
            V.memset(sO[:, 8, :], C1H)
            V.memset(sO[:, 9, :], C1L)
            V.memset(sG[:, 8:10, :], 0.0)

            # ---------------- phase-B critical chain (issue first!) ----------
            # a = roll(gc, 1) built in SBUF
            V.tensor_copy(a_i[:, 1:128, :], gc_i[:, 0:127, :])
            V.tensor_copy(a_i[:, 0:1, :], gc_i[:, 127:128, :])
            d_i = prep.tile([32, 128, 2], FP, tag="d_i")
            V.tensor_tensor(d_i[:], gc_i[:], a_i[:], Alu.subtract)
            dsq = prep.tile([32, 128, 2], FP, tag="dsq")
            Gp.tensor_tensor(dsq[:], d_i[:], d_i[:], Alu.mult)
            e = prep.tile([32, 128], FP, tag="e")
            Gp.tensor_tensor(e[:], dsq[:, :, 0], dsq[:, :, 1], Alu.add)
            einv = prep.tile([32, 128], FP, tag="einv")
            V.reciprocal(einv[:], e[:])
            asq = prep.tile([32, 128, 2], FP, tag="asq")
            Gp.tensor_tensor(asq[:], a_i[:], a_i[:], Alu.mult)
            zA = prep.tile([32, 128], FP, tag="zA")
            Gp.tensor_tensor(zA[:], asq[:, :, 0], asq[:, :, 1], Alu.add)
            t_ad = prep.tile([32, 128, 2], FP, tag="t_ad")
            Gp.tensor_tensor(t_ad[:], a_i[:], d_i[:], Alu.mult)
            a2 = prep.tile([32, 128], FP, tag="a2")
            Gp.tensor_tensor(a2[:], t_ad[:, :, 0], t_ad[:, :, 1], Alu.add)
            r_01 = prep.tile([32, 128, 2], FP, tag="r_01")
            V.scalar_tensor_tensor(r_01[:, :, 0], d_i[:, :, 0], 10.0, einv[:], Alu.mult, Alu.mult)
            V.scalar_tensor_tensor(r_01[:, :, 1], d_i[:, :, 1], 10.0, einv[:], Alu.mult, Alu.mult)
            r2 = prep.tile([32, 128], FP, tag="r2")
            V.scalar_tensor_tensor(r2[:], a2[:], -10.0, einv[:], Alu.mult, Alu.mult)


            def split_pair(dst, src_, s_hi, s_lo):
                # contiguous bf16 writes; channel-major strided reads
                srcT = src_.rearrange("g q c -> g c q")
                S.activation(dst[:, s_hi:s_hi + 2, :], srcT, Act.Copy)
                if s_lo is not None:
                    V.tensor_tensor(dst[:, s_lo:s_lo + 2, :], srcT,
                                    dst[:, s_hi:s_hi + 2, :], Alu.subtract)

            def split_one(dst, src_, s_hi, s_lo):
                S.activation(dst[:, s_hi, :], src_, Act.Copy)
                if s_lo is not None:
                    V.tensor_tensor(dst[:, s_lo, :], src_, dst[:, s_hi, :], Alu.subtract)

            # ---- phase A (pc/gk): PK + Q slabs ----
            q_01 = prep.tile([32, 128, 2], FP, tag="q_01")
            S.activation(q_01[:], pc_i[:], Act.Copy, scale=2.0 * SC)
            psq = prep.tile([32, 128, 2], FP, tag="psq")
            Gp.tensor_tensor(psq[:], pc_i[:], pc_i[:], Alu.mult)
            zP = prep.tile([32, 128], FP, tag="zP")
            Gp.tensor_tensor(zP[:], psq[:, :, 0], psq[:, :, 1], Alu.add)
            q2 = prep.tile([32, 128], FP, tag="q2")
            S.activation(q2[:], zP[:], Act.Copy, scale=-SC)
            split_pair(sPK, pc_i[:], P0 + 0, P0 + 2)
            split_pair(sPK, gk_i[:], K0 + 0, K0 + 2)
            V.tensor_copy(sPK[:, P0 + 4:P0 + 6, :], sPK[:, P0 + 0:P0 + 2, :])
            V.tensor_copy(sPK[:, K0 + 4:K0 + 6, :], sPK[:, K0 + 0:K0 + 2, :])
            split_pair(sQ, q_01[:], 0, 4)
            split_one(sQ, q2[:], 6, 7)
            V.tensor_copy(sQ[:, 2:4, :], sQ[:, 0:2, :])

            # ---- E slab ----
            er = prep.tile([32, 128], FP, tag="er")
            S.activation(er[:], e[:], Act.Copy, scale=-SC / 100.0)
            split_one(sE, er[:], 6, 7)

            # ---- O slab ----
            o_01 = prep.tile([32, 128, 2], FP, tag="o_01")
            S.activation(o_01[:], a_i[:], Act.Copy, scale=2.0 * SC)
            o2 = prep.tile([32, 128], FP, tag="o2")
            S.activation(o2[:], zA[:], Act.Copy, scale=-SC)
            split_pair(sO, o_01[:], 0, 4)
            split_one(sO, o2[:], 6, 7)
            V.tensor_copy(sO[:, 2:4, :], sO[:, 0:2, :])

            # ---- G slab ----
            split_pair(sG, r_01[:], 0, 4)
            split_one(sG, r2[:], 6, 7)
            V.tensor_copy(sG[:, 2:4, :], sG[:, 0:2, :])

            # ---------------- DRAM bounce + bulk reloads ---------------------
            # slabALL r-slots: 0 = G, 1 = O, 2 = Q, 3 = E (matches rhsA r)
            slabPK_d = nc.dram_tensor("slabPK", [32, 20, 128], BF)
            slabALL_d = nc.dram_tensor("slabALL", [32, 4, 10, 128], BF)
            slabI_d = nc.dram_tensor("slabI", [4, 8, 4, 10, 128], BF)
            nc.sync.dma_start(slabPK_d[:, :, :], sPK[:])
            nc.scalar.dma_start(slabALL_d[:, 2, :, :], sQ[:])
            nc.sync.dma_start(slabALL_d[:, 3, :, :], sE[:])
            nc.scalar.dma_start(slabALL_d[:, 1, :, :], sO[:])
            nc.sync.dma_start(slabALL_d[:, 0, :, :], sG[:])
            # DRAM->DRAM per-instance-slot relayout (fixes the g-stride so the
            # final reload collapses to one 3-D DMA per slot)
            for i in range(BG):
                eng = nc.sync if i % 2 == 0 else nc.scalar
                eng.dma_start(slabI_d[i], slabALL_d[i:32:4, :, :, :])

            for i in range(BG):
                for s in range(2):
                    eng = nc.sync if (i + s) % 2 == 0 else nc.scalar
                    eng.dma_start(
                        lhsA[10 * i:10 * i + 10, s, :, :],
                        slabPK_d[i:32:4, 10 * s:10 * s + 10, :]
                        .rearrange("b t p -> t b p"),
                    )
            for i in range(BG):
                eng = nc.sync if i % 2 == 0 else nc.scalar
                eng.dma_start(
                    rhsA[10 * i:10 * i + 10, :, :, 128 * i:128 * (i + 1)],
                    slabI_d[i].rearrange("b r t p -> t (b r) p"),
                )

            # ---------------- constants (issue late; needed late) ------------
            iotaF = cpool.tile([128, 128], FP, tag="iotaF")
            Gp.iota(iotaF[:], pattern=[[1, 128]], channel_multiplier=0,
                    allow_small_or_imprecise_dtypes=True)
            iotaC = cpool.tile([128, 1], FP, tag="iotaC")
            Gp.iota(iotaC[:], pattern=[[0, 1]], channel_multiplier=1,
                    allow_small_or_imprecise_dtypes=True)
            iotaB4 = cpool.tile([128, BG, 128], FP, tag="iotaB4")
            Gp.iota(iotaB4[:], pattern=[[0, BG], [1, 128]], channel_multiplier=0,
                    allow_small_or_imprecise_dtypes=True)
            onesc = cpool.tile([128, 1], FP, tag="onesc")
            Gp.memset(onesc[:], 1.0)

            # ---- table region (Vector copies; keep ACT queue free) ----
            V.tensor_copy(sTB[:, 0:2, :], a_i[:].rearrange("g q c -> g c q"))
            V.tensor_copy(sTB[:, 2:4, :], d_i[:].rearrange("g q c -> g c q"))
            V.tensor_copy(sTB[:, 12:14, :], po_i[:].rearrange("g q c -> g c q"))
            V.tensor_copy(sTB[:, 4:6, :], sG[:, 0:5:4, :])
            V.tensor_copy(sTB[:, 6:8, :], sG[:, 1:6:4, :])
            V.tensor_copy(sTB[:, 8:10, :], sG[:, 6:8, :])
            V.tensor_copy(sTB[:, 10:12, :], sPK[:, P0 + 0:P0 + 2, :])
            V.tensor_copy(sTB[:, 14:16, :], sPK[:, K0 + 0:K0 + 2, :])
            V.tensor_copy(sTB[:, 16, :], mk_i[:])

            # ---------------- gather tables (one batched XBAR transpose) -----
            # in (32, 17*128) -> out stgB[n, j, g] = sTB[g, j, n]
            # gather matmuls read stgB[:, j, g] directly; the tail reads its
            # transposed per-point inputs (px..mk, rows 10-16) as bf16 slices.
            stgB = keep.tile([128, 17, 32], BF, tag="stgB")
            nc.scalar.dma_start_transpose(
                stgB[:], sTB[:].rearrange("g j n -> g (j n)"))

            exA = keep.tile([128, 16, 14], FP, tag="exA")
            exB = keep.tile([128, 16, 14], FP, tag="exB")
            tl = prep

            def tail_chunk(ex, gl, sfx, dense=False):
                gs = slice(gl, gl + 16)

                def TT(name, a, bb, op, eng=V):
                    if dense:
                        eng = V
                    r = tl.tile([128, 16], FP, tag=sfx + name)
                    eng.tensor_tensor(r[:], a, bb, op)
                    return r

                r0 = TT("r0", ex[:, :, 4], ex[:, :, 5], Alu.add)
                r1 = TT("r1", ex[:, :, 6], ex[:, :, 7], Alu.add, Gp)
                r2t = TT("r2t", ex[:, :, 8], ex[:, :, 9], Alu.add)
                v1 = TT("v1", stgB[:, 10, gs], r0[:], Alu.mult, Gp)
                v2 = TT("v2", stgB[:, 11, gs], r1[:], Alu.mult)
                gst = TT("gst", v1[:], v2[:], Alu.add, Gp)
                gst = TT("gst2", gst[:], r2t[:], Alu.add)
                c2t = tl.tile([128, 16], FP, tag=sfx + "c2t")
                S.activation(c2t[:], gst[:], Act.Copy, bias=M23)
                c3t = tl.tile([128, 16], FP, tag=sfx + "c3t")
                S.activation(c3t[:], c2t[:], Act.Relu, bias=m23n[:])
                tst = tl.tile([128, 16], FP, tag=sfx + "tst")
                V.tensor_scalar(tst[:], c3t[:], 9.0, None, Alu.min)
                m1 = TT("m1", tst[:], ex[:, :, 2], Alu.mult, Gp)
                tgx = tl.tile([128, 16], FP, tag=sfx + "tgx")
                V.scalar_tensor_tensor(tgx[:], m1[:], 0.1, ex[:, :, 0], Alu.mult, Alu.add)
                m2 = TT("m2", tst[:], ex[:, :, 3], Alu.mult, Gp)
                tgy = tl.tile([128, 16], FP, tag=sfx + "tgy")
                V.scalar_tensor_tensor(tgy[:], m2[:], 0.1, ex[:, :, 1], Alu.mult, Alu.add)

                def smooth_l1_sum(pred_x, pred_y, tx, ty, px_, py_, name,
                                  mask=None):
                    parts = []
                    for ci, (pr, tt_, pp) in enumerate(((pred_x, tx, px_), (pred_y, ty, py_))):
                        s2fx = sfx + name + str(ci)
                        e1 = TT(name + str(ci) + "e1", tt_, pp, Alu.subtract, Gp)
                        dfe = tl.tile([128, 16], FP, tag=s2fx + "dfe")
                        V.scalar_tensor_tensor(dfe[:], e1[:], -0.25, pr, Alu.mult, Alu.add)
                        ad = tl.tile([128, 16], FP, tag=s2fx + "ad")
                        S.activation(ad[:], dfe[:], Act.Abs)
                        m = tl.tile([128, 16], FP, tag=s2fx + "m")
                        V.tensor_scalar(m[:], ad[:], BETA, None, Alu.min)
                        uu = tl.tile([128, 16], FP, tag=s2fx + "u")
                        V.scalar_tensor_tensor(uu[:], m[:], -0.5, ad[:], Alu.mult, Alu.add)
                        parts.append((m, uu))
                    sl0 = tl.tile([128, 16], FP, tag=sfx + name + "sl0")
                    V.scalar_tensor_tensor(sl0[:], parts[0][0][:], 4.0, parts[0][1][:],
                                           Alu.mult, Alu.mult)
                    sl1 = tl.tile([128, 16], FP, tag=sfx + name + "sl1")
                    V.scalar_tensor_tensor(sl1[:], parts[1][0][:], 4.0, parts[1][1][:],
                                           Alu.mult, Alu.mult)
                    acc = TT(name + "acc", sl0[:], sl1[:], Alu.add, Gp)
                    if mask is not None:
                        acc = TT(name + "accm", acc[:], mask, Alu.mult)
                    r_ = tl.tile([128, 1], FP, tag=sfx + name + "r")
                    V.tensor_reduce(r_[:], acc[:], Ax.X, Alu.add)
                    return r_

                s1r = smooth_l1_sum(stgB[:, 12, gs], stgB[:, 13, gs], tgx[:], tgy[:],
                                    stgB[:, 10, gs], stgB[:, 11, gs], "i1")
                s2r = smooth_l1_sum(ex[:, :, 12], ex[:, :, 13], stgB[:, 14, gs], stgB[:, 15, gs],
                                    ex[:, :, 10], ex[:, :, 11], "i2",
                                    mask=stgB[:, 16, gs])
                return s1r, s2r

            ps_grid_cm = tc.tile_pool(name="ps_grid", bufs=2, space="PSUM")
            ps_e_cm = tc.tile_pool(name="ps_e", bufs=1, space="PSUM")
            ps_d2_cm = tc.tile_pool(name="ps_d2", bufs=1, space="PSUM")
            ps_ex_cm = tc.tile_pool(name="ps_ex", bufs=2, space="PSUM")
            ps_grid = ps_grid_cm.__enter__()
            ps_e = ps_e_cm.__enter__()
            ps_d2 = ps_d2_cm.__enter__()
            ps_ex = ps_ex_cm.__enter__()

            # ---------------- main loop ----------------
            for b in range(NB):
                g0 = b * BG
                gps = ps_grid.tile([128, BG, 128], FP, tag="gps")
                ops = ps_grid.tile([128, BG, 128], FP, tag="ops")
                erep = ps_e.tile([128, BG, 128], FP, tag="erep")
                d2ps = ps_d2.tile([128, BG, 128], FP, tag="d2ps")
                gv = gps[:].rearrange("p i n -> p (i n)")
                ov = ops[:].rearrange("p i n -> p (i n)")
                ev = erep[:].rearrange("p i n -> p (i n)")
                dv = d2ps[:].rearrange("p i n -> p (i n)")
                lp = lhsA[:, 0, b, :]
                nc.tensor.matmul(gv, lp, rhsA[:, b, 0, :], start=True, stop=True)
                nc.tensor.matmul(dv, lhsA[:, 1, b, :], rhsA[:, b, 2, :], start=True, stop=True)
                nc.tensor.matmul(ev, lp, rhsA[:, b, 3, :], start=True, stop=True)
                nc.tensor.matmul(ov, lp, rhsA[:, b, 1, :], start=True, stop=True)

                # -------- item1: t = clamp(round(g'), 0, 9) --------
                s2t = main.tile([128, BG, 128], FP, tag="s2t")
                S.activation(s2t[:], gps[:], Act.Copy, bias=M23)
                s3t = main.tile([128, BG, 128], BF, tag="s3t")
                S.activation(s3t[:], s2t[:], Act.Relu, bias=m23n[:])
                t = main.tile([128, BG, 128], BF, tag="t")
                V.tensor_scalar(t[:], s3t[:], 9.0, None, Alu.min)
                hq = main.tile([128, BG, 128], BF, tag="hq")
                V.scalar_tensor_tensor(hq[:], gps[:], -2.0, t[:], Alu.mult, Alu.add)
                q = main.tile([128, BG, 128], BF, tag="q")
                Gp.tensor_tensor(q[:], hq[:], t[:], Alu.mult)
                erepS = main.tile([128, BG, 128], FP, tag="erepS")
                S.activation(erepS[:], erep[:], Act.Copy)
                vE = main.tile([128, BG, 128], FP, tag="vE")
                Gp.tensor_tensor(vE[:], q[:], erepS[:], Alu.mult)
                dqA = main.tile([128, BG, 128], FP, tag="dqA")
                V.tensor_tensor(dqA[:], vE[:], ops[:], Alu.add)
                pkN = main.tile([128, BG, 128], FP, tag="pkN")
                V.scalar_tensor_tensor(pkN[:], dqA[:], CQ, iotaB4[:],
                                       Alu.subtract, Alu.subtract)

                # -------- item2 --------
                dq2 = main.tile([128, BG, 128], FP, tag="dq2")
                S.activation(dq2[:], d2ps[:], Act.Copy, bias=-CQ)
                pk2 = main.tile([128, BG, 128], FP, tag="pk2")
                Gp.tensor_tensor(pk2[:], dq2[:], iotaB4[:], Alu.subtract)

                mx = main.tile([128, BG], FP, tag="mx")
                V.tensor_reduce(mx[:], pkN[:], Ax.X, Alu.max)
                mx2 = main.tile([128, BG], FP, tag="mx2")
                V.tensor_reduce(mx2[:], pk2[:], Ax.X, Alu.max)
                mxb1 = main.tile([128, BG], FP, tag="mxb1")
                S.activation(mxb1[:], mx[:], Act.Copy, scale=-1.0, bias=1.0)
                mxb2 = main.tile([128, BG], FP, tag="mxb2")
                S.activation(mxb2[:], mx2[:], Act.Copy, scale=-1.0, bias=1.0)

                oh = main.tile([128, BG, 128], BF, tag="oh")
                oh2 = main.tile([128, BG, 128], BF, tag="oh2")
                for i in range(BG):
                    if i < 2:
                        S.activation(oh[:, i, :], pkN[:, i, :], Act.Relu, bias=mxb1[:, i:i + 1])
                        V.tensor_scalar(oh2[:, i, :], pk2[:, i, :], mx2[:, i:i + 1], None, Alu.is_equal)
                    else:
                        V.tensor_scalar(oh[:, i, :], pkN[:, i, :], mx[:, i:i + 1], None, Alu.is_equal)
                        S.activation(oh2[:, i, :], pk2[:, i, :], Act.Relu, bias=mxb2[:, i:i + 1])

                # -------- XBAR-transpose one-hots, gather via matmul --------
                ohT = main.tile([128, BG, 128], BF, tag="ohT")
                oh2T = main.tile([128, BG, 128], BF, tag="oh2T")
                nc.sync.dma_start_transpose(ohT[:], oh[:].rearrange("m i n -> m (i n)"))
                nc.sync.dma_start_transpose(oh2T[:], oh2[:].rearrange("m i n -> m (i n)"))

                exPS = ps_ex.tile([128, BG, 14], FP, tag="exPS")
                for i in range(BG):
                    g = g0 + i
                    nc.tensor.matmul(exPS[:, i, 0:10], ohT[:, i, :],
                                     stgB[:, 0:10, g], start=True, stop=True)
                    nc.tensor.matmul(exPS[:, i, 10:14], oh2T[:, i, :],
                                     stgB[:, 10:14, g], start=True, stop=True)
                ext = exA if b < 4 else exB
                S.activation(ext[:, (g0 % 16):(g0 % 16) + BG, :], exPS[:], Act.Copy)
                if b == 3:
                    _TAILA = tail_chunk(exA, 0, "A")

            # ---------------- tail (chunked, overlaps main loop) -------------
            s1a, s2a = _TAILA
            s1b, s2b = tail_chunk(exB, 16, "B", dense=True)
            sboth = tl.tile([128, 2], FP, tag="sboth")
            V.tensor_tensor(sboth[:, 0:1], s1a[:], s1b[:], Alu.add)
            V.tensor_tensor(sboth[:, 1:2], s2a[:], s2b[:], Alu.add)
            ps_ex_cm.__exit__(None, None, None)
            ps_d2_cm.__exit__(None, None, None)
            ps_e_cm.__exit__(None, None, None)
            ps_out_cm = tc.tile_pool(name="ps_out", bufs=1, space="PSUM")
            ps_out = ps_out_cm.__enter__()
            sc_ps = ps_out.tile([2, 1], FP, tag="sc_ps")
            nc.tensor.matmul(sc_ps[:], sboth[:], onesc[:], start=True, stop=True)
            outsb = tl.tile([2, 1], FP, tag="outsb")
            V.tensor_copy(outsb[:], sc_ps[:])
            nc.sync.dma_start(out_d[:].rearrange("(a b) -> a b", b=1), outsb[:])
            ps_out_cm.__exit__(None, None, None)
            ps_grid_cm.__exit__(None, None, None)

    return nc


_CACHE = {}


def _get_program():
    if "nc" not in _CACHE:
        nc = bacc.Bacc("TRN2", target_bir_lowering=False, num_devices=N_CORES)
        pc_d = nc.declare_dram_parameter("pc", [G, P, 2], dt.float32, isOutput=False)
        po_d = nc.declare_dram_parameter("po", [G, P, 2], dt.float32, isOutput=False)
        gc_d = nc.declare_dram_parameter("gc", [G, P, 2], dt.float32, isOutput=False)
        gk_d = nc.declare_dram_parameter("gk", [G, P, 2], dt.float32, isOutput=False)
        mk_d = nc.declare_dram_parameter("mk", [G, P], dt.float32, isOutput=False)
        out_d = nc.declare_dram_parameter("out", [2], dt.float32, isOutput=True)
        _build(nc, pc_d[:], po_d[:], gc_d[:], gk_d[:], mk_d[:], out_d[:])
        nc.compile()
        _CACHE["nc"] = nc
    return _CACHE["nc"]


def _in_maps(inputs):
    pc = np.ascontiguousarray(inputs["pred_contours"], dtype=np.float32)
    po = np.ascontiguousarray(inputs["pred_offsets"], dtype=np.float32)
    gc = np.ascontiguousarray(inputs["gt_contours"], dtype=np.float32)
    gk = np.ascontiguousarray(inputs["gt_key_points"], dtype=np.float32)
    mk = np.ascontiguousarray(inputs["gt_key_points_mask"]).astype(np.float32)
    maps = []
    for c in range(N_CORES):
        s = slice(c * G, (c + 1) * G)
        maps.append({
            "pc": pc[s], "po": po[s], "gc": gc[s], "gk": gk[s], "mk": mk[s],
        })
    return maps


def kernel(pred_contours, pred_offsets, gt_contours, gt_key_points, gt_key_points_mask,
           _results_hook=None):
    inputs = {
        "pred_contours": pred_contours,
        "pred_offsets": pred_offsets,
        "gt_contours": gt_contours,
        "gt_key_points": gt_key_points,
        "gt_key_points_mask": gt_key_points_mask,
    }
    nc = _get_program()
    res = run_bass_kernel_spmd(nc, _in_maps(inputs), list(range(N_CORES)))
    if _results_hook is not None:
        _results_hook(res)
    s1 = f32(0.0)
    s2 = f32(0.0)
    for r in res.results:
        s1 = f32(s1 + f32(r["out"][0]))
        s2 = f32(s2 + f32(r["out"][1]))
    cnt1 = f32(N * P * 2)
    cnt2 = f32(max(float(np.sum(gt_key_points_mask != 0)) * 2.0, 1.0))
    loss = f32(f32(s1 / cnt1) * f32(0.5) + f32(s2 / cnt2) * f32(0.5))
    return np.asarray(loss, dtype=np.float32)
